# revision 10
# baseline (speedup 1.0000x reference)
"""Baichuan attention decode step on 8 Trainium2 NeuronCores (Bass/Tile).

Head-sharded tensor parallel: 40 heads -> 5 heads per core.
Per core:
  - QKV projection for own heads:  qkv[8,1920] = hs[8,5120] @ Wsh.T
    (Wsh = own-head rows of W_pack, q-rows pre-scaled by 1/sqrt(128))
  - scores (transposed): sT[pos,tok] per head via PE over the OLD k cache,
    + mask, exp on ACT. Winner columns (cache slots being overwritten)
    carry -1e30 in the shipped mask, so their stale-k exp terms vanish.
  - the 8 new tokens' scores come from a rank-8 side path:
    corr[j,h,t] = exp(k_new_j . q_t + mask[j,t]) (losers of duplicate
    positions get -1e30 -> 0). corr feeds both the denominator (ones8
    matmul) and the attention numerator (x v_new).
  - softmax denominator via ones-matmul + strided DVE reduce; division
    applied to the attention output (scale-after-matmul)
  - o_proj partial: out[8,5120] = attn[8,640] @ W_o[:,own_cols].T
Host sums the 8 partial outputs (the "all-reduce").

Only the 8 mask rows at input_pos are shipped to the device (gathered on
host), and input_pos is baked into the compiled program (recompiled per
distinct input_pos, cached).

All HBM-resident streams (W_pack, W_o, k/v caches, hidden states) are
fp16; matmuls accumulate in fp32 PSUM; softmax bookkeeping stays fp32.

DMA plan (the kernel is DMA-bound; HWDGE descriptor generation costs
~0.7-1.3us per dma_start, serial per ring, so starts are batched):
  scalar ring: hsT(+ones column), W_pack groups 0-1, one packed
    constants blob (maskT/maskN/eye8/ones/ones8), then the per-piece
    output stores at the end.
  sync ring: W_pack groups 2+, k cache (1 start), v cache (1 start),
    W_o in 10 pieces (o_proj rides them; only the last piece's chain
    is on the critical tail).
"""

import os
import sys
import math
from contextlib import ExitStack

import numpy as np

for _p in ("/opt/trn_rl_repo", "/opt/trn_rl_repo/concourse"):
    if os.path.isdir(_p) and _p not in sys.path:
        sys.path.insert(0, _p)

import concourse.tile as tile  # noqa: E402
from concourse import bacc, mybir  # noqa: E402
from concourse.bass_utils import run_bass_kernel_spmd  # noqa: E402

F32 = mybir.dt.float32
F16 = mybir.dt.float16

HIDDEN = 5120
NH = 40
HD = 128
L = 2048
Q = 8
NCORES = 8
HPC = NH // NCORES          # 5 heads per core
KC = HIDDEN // 128          # 40 contraction chunks
MQKV = 3 * HPC * HD         # 1920 qkv output dim per core
NPOS = L // 128             # 16 position chunks
WP_G = 4                    # wpT k-chunks per DMA group
WO_P = 512                  # o_proj N-piece size (1 PSUM bank)
NPIECE = HIDDEN // WO_P     # 10

# constants blob layout (fp32, [128, CB_N]):
CB_MASKT = 0                          # [128, 640] maskT
CB_EYE8 = CB_MASKT + HPC * NPOS * Q   # [0:8, 640:648] eye8
CB_MASKN = CB_EYE8 + Q                # [0:8, 648:688] maskN
CB_ONES = CB_MASKN + HPC * Q          # [0:1, 688:816] ones row
CB_ONES8 = CB_ONES + 128              # [0:8, 816:817] ones8 col
CB_N = CB_ONES8 + 1

_CACHE = {}


def _build_program(pos, winners):
    """Build the SPMD Bass program with input_pos baked in.

    pos: list of 8 ints. winners: list of bools (True if token t's cache
    write survives, i.e. it is the last occurrence of that position).
    """
    nc = bacc.Bacc("TRN2", target_bir_lowering=False, debug=False)

    hsT_d = nc.dram_tensor("hsT", [128, KC * Q + 1], F16, kind="ExternalInput")
    wpT_d = nc.dram_tensor("wpT", [128, KC, MQKV], F16, kind="ExternalInput")
    kcT_d = nc.dram_tensor("kcT", [128, HPC, L], F16, kind="ExternalInput")
    vc_d = nc.dram_tensor("vc", [128, HPC, NPOS, HD], F16, kind="ExternalInput")
    cb_d = nc.dram_tensor("cb", [128, CB_N], F32, kind="ExternalInput")
    wo_d = nc.dram_tensor("wo", [128, NPIECE, HPC, WO_P], F16, kind="ExternalInput")
    out_d = nc.dram_tensor("out", [Q, HIDDEN], F32, kind="ExternalOutput")

    with tile.TileContext(nc) as tc, ExitStack() as ctx:
        sb = ctx.enter_context(tc.tile_pool(name="sb", bufs=1))
        ps = ctx.enter_context(tc.tile_pool(name="ps", bufs=1, space="PSUM"))

        # ---- big stream all on the sync ring in strict consumption order
        # (a second ring's transfers get starved behind this one's queued
        # descriptors, so only tiny/late data rides the scalar ring: hsT
        # lands in the preamble window before the sync stream saturates) ----
        hsTt = sb.tile([128, KC * Q + 1], F16, tag="hsT")
        nc.scalar.dma_start(hsTt[:], hsT_d.ap())
        hsT = hsTt[:, 0:KC * Q].rearrange("p (k t) -> p k t", k=KC)
        ones_r = hsTt[:, KC * Q:KC * Q + 1]           # fp16 ones column

        # W_pack chunk 0 also rides the scalar ring: it transfers during
        # the sync ring's descriptor-gen window, so engines never idle at
        # the start. Chunks 1+ stream on sync in consumption order.
        groups = [1, 1, 2] + [WP_G] * ((KC - 4) // WP_G)
        assert sum(groups) == KC
        wp_tiles = []
        g0 = 0
        for gi, gn in enumerate(groups):
            wp = sb.tile([128, WP_G, MQKV], F16, tag="wstream", bufs=3)
            eng = nc.scalar if gi == 0 else nc.sync
            eng.dma_start(wp[:, 0:gn, :], wpT_d.ap()[:, g0:g0 + gn, :])
            wp_tiles.append((g0, gn, wp))
            g0 += gn

        # constants blob on the scalar ring (needed mid-kernel; off the
        # critical stream)
        cb = sb.tile([128, CB_N], F32, tag="cb")
        nc.scalar.dma_start(cb[:], cb_d.ap())
        maskT = cb[:, CB_MASKT:CB_MASKT + HPC * NPOS * Q].rearrange(
            "p (h c t) -> p h c t", h=HPC, c=NPOS)
        eye8 = cb[0:Q, CB_EYE8:CB_EYE8 + Q]
        maskN = cb[0:Q, CB_MASKN:CB_MASKN + HPC * Q].rearrange(
            "p (h t) -> p h t", h=HPC)
        ones_row = cb[0:1, CB_ONES:CB_ONES + 128]
        ones8 = cb[0:Q, CB_ONES8:CB_ONES8 + 1]

        # ---- big stream on the sync ring in strict consumption order ----
        kcT = sb.tile([128, HPC, L], F16, tag="kcT")
        nc.sync.dma_start(kcT[:], kcT_d.ap())
        vc = sb.tile([128, HPC, NPOS, HD], F16, tag="vc")
        nc.sync.dma_start(vc[:], vc_d.ap())
        # W_o fully resident: its stream overlaps the scores/attention
        # phase and o_proj consumes it piece by piece as it lands.
        wo_sb = sb.tile([128, NPIECE, HPC, WO_P], F16, tag="wo")
        for piece in range(NPIECE):
            nc.sync.dma_start(wo_sb[:, piece], wo_d.ap()[:, piece, :, :])

        # ---- QKV projection: qkv[8, 1920] ----
        ps_qkv = ps.tile([Q, MQKV], F32, tag="A")
        nslices = [(0, 512), (512, 1024), (1024, 1536), (1536, MQKV)]
        for (g0, gn, wp) in wp_tiles:
            for i in range(gn):
                kc = g0 + i
                for (n0, n1) in nslices:
                    nc.tensor.matmul(
                        ps_qkv[0:Q, n0:n1],
                        hsT[:, kc, :],
                        wp[:, i, n0:n1],
                        start=(kc == 0),
                        stop=(kc == KC - 1),
                    )
        # q block first: it alone gates the scores matmuls
        qkv = sb.tile([Q, MQKV], F32, tag="qkvsb")
        nc.vector.tensor_copy(qkv[0:Q, 0:HPC * HD], ps_qkv[0:Q, 0:HPC * HD])

        # ---- transposes: qT per head, then scores ----
        qT = sb.tile([128, HPC, Q], F16, tag="qT")
        tq = ps.tile([128, HPC, Q], F32, tag="S1", bufs=2)
        for h in range(HPC):
            nc.tensor.transpose(tq[:, h, :], qkv[0:Q, h * HD:(h + 1) * HD], eye8)
        nc.vector.tensor_copy(qT[:], tq[:])

        # scores (transposed) over the OLD cache; winner columns are
        # masked to -1e30 host-side so their stale-k terms exp to 0.
        ps_sc = ps.tile([128, HPC, NPOS, Q], F32, tag="A")
        for h in range(HPC):
            for cj in range(NPOS):
                nc.tensor.matmul(
                    ps_sc[:, h, cj, :],
                    kcT[:, h, cj * 128:(cj + 1) * 128],
                    qT[:, h, :],
                    start=True,
                    stop=True,
                )

        # k/v blocks of qkv; kT_new transposes (PE order: after scores)
        nc.vector.tensor_copy(qkv[0:Q, HPC * HD:], ps_qkv[0:Q, HPC * HD:])
        tk = ps.tile([128, HPC, Q], F32, tag="S1", bufs=2)
        for h in range(HPC):
            nc.tensor.transpose(
                tk[:, h, :], qkv[0:Q, HPC * HD + h * HD:HPC * HD + (h + 1) * HD], eye8
            )

        # ---- + mask, exp (batched over heads) ----
        scT = sb.tile([128, HPC, NPOS, Q], F32, tag="scT")
        nc.vector.tensor_add(scT[:], ps_sc[:], maskT)
        expT = sb.tile([128, HPC, NPOS, Q], F16, tag="expT")
        nc.scalar.activation(expT[:], scT[:], mybir.ActivationFunctionType.Exp)

        # ---- corr = exp(k_new . q + maskN) (rank-8 side path) ----
        ktn = sb.tile([128, HPC, Q], F16, tag="ktn")
        nc.vector.tensor_copy(ktn[:], tk[:])
        corr_ps = ps.tile([Q, HPC, Q], F32, tag="S1", bufs=2)
        for h in range(HPC):
            nc.tensor.matmul(
                corr_ps[0:Q, h, :], ktn[:, h, :], qT[:, h, :], start=True, stop=True
            )
        corr_s = sb.tile([Q, HPC, Q], F32, tag="corrs")
        nc.vector.tensor_add(corr_s[:], corr_ps[:], maskN)
        corr = sb.tile([Q, HPC, Q], F32, tag="corr")
        nc.scalar.activation(corr[:], corr_s[:], mybir.ActivationFunctionType.Exp)

        # ---- denominators ----
        sums = sb.tile([1, HPC, Q], F32, tag="sums")
        for h in range(HPC):
            ps_sum = ps.tile([1, NPOS, Q], F32, tag="S1", bufs=2)
            nc.tensor.matmul(
                ps_sum[0:1],
                ones_r,
                expT[:, h].rearrange("p c t -> p (c t)"),
                start=True,
                stop=True,
            )
            nc.vector.tensor_reduce(
                sums[0:1, h, :],
                ps_sum.rearrange("p c t -> p t c"),
                axis=mybir.AxisListType.X,
                op=mybir.AluOpType.add,
            )
        # + the new tokens' exp terms
        ps_cs = ps.tile([1, HPC, Q], F32, tag="S1", bufs=2)
        nc.tensor.matmul(
            ps_cs[0:1],
            ones8,
            corr.rearrange("j h t -> j (h t)"),
            start=True,
            stop=True,
        )
        sums_f = sb.tile([1, HPC, Q], F32, tag="sumsf")
        nc.vector.tensor_add(sums_f[:], sums[:], ps_cs[:])

        # ---- attention numerator ----
        ps_at = ps.tile([128, HPC, Q], F32, tag="S1", bufs=2)
        for h in range(HPC):
            for cj in range(NPOS):
                nc.tensor.matmul(
                    ps_at[:, h, :],
                    vc[:, h, cj, :],
                    expT[:, h, cj, :],
                    start=(cj == 0),
                    stop=False,
                )
            # + rank-8 correction with the new v rows (v_new = qkv v-slice)
            nc.tensor.matmul(
                ps_at[:, h, :],
                qkv[0:Q, 2 * HPC * HD + h * HD:2 * HPC * HD + (h + 1) * HD],
                corr[:, h, :],
                start=False,
                stop=True,
            )

        # ---- reciprocals -> broadcast over partitions ----
        recip = sb.tile([1, HPC * Q], F32, tag="recip")
        nc.vector.reciprocal(recip[:], sums_f.rearrange("p h t -> p (h t)"))
        ps_bc = ps.tile([128, HPC * Q], F32, tag="S1", bufs=2)
        nc.tensor.matmul(ps_bc[:], ones_row, recip[0:1, :], start=True, stop=True)
        bc = sb.tile([128, HPC * Q], F32, tag="bcsb")
        nc.vector.tensor_copy(bc[:], ps_bc[:])
        attn = sb.tile([128, HPC * Q], F16, tag="attn")
        nc.vector.tensor_mul(attn[:], ps_at.rearrange("p h t -> p (h t)"), bc[:])

        # ---- o_proj partial: out[8, 5120], shipped out per piece on the
        # scalar ring so only the last piece's store is on the tail ----
        out_sb = sb.tile([Q, HIDDEN], F32, tag="outsb")
        for piece in range(NPIECE):
            ps_o = ps.tile([Q, WO_P], F32, tag="PO", bufs=2)
            for h in range(HPC):
                nc.tensor.matmul(
                    ps_o[0:Q, :],
                    attn[:, h * Q:(h + 1) * Q],
                    wo_sb[:, piece, h, :],
                    start=(h == 0),
                    stop=(h == HPC - 1),
                )
            n0 = piece * WO_P
            nc.vector.tensor_copy(out_sb[0:Q, n0:n0 + WO_P], ps_o[:])
            if piece % 2 == 1:
                m0 = (piece - 1) * WO_P
                nc.scalar.dma_start(
                    out_d.ap()[:, m0:m0 + 2 * WO_P], out_sb[0:Q, m0:m0 + 2 * WO_P])

    nc.compile()
    return nc


def _get_program(pos, winners):
    key = (tuple(pos), tuple(winners))
    if key not in _CACHE:
        _CACHE[key] = _build_program(pos, winners)
    return _CACHE[key]


def _prep_inputs(input_pos, hidden_states, attention_mask, W_pack, W_o,
                 k_cache, v_cache):
    """Host-side sharding: returns (in_maps, pos, winners)."""
    pos = [int(p) for p in np.asarray(input_pos).reshape(-1)]
    last = {}
    for t, p in enumerate(pos):
        last[p] = t
    winners = [last[p] == t for t, p in enumerate(pos)]

    hs = np.asarray(hidden_states, dtype=np.float32).reshape(Q, HIDDEN)
    # hsT[p, kc*8+t] = hs[t, kc*128+p]; final column = fp16 ones
    hsT = np.ones((128, KC * Q + 1), dtype=np.float16)
    hsT[:, 0:KC * Q] = hs.T.reshape(KC, 128, Q).transpose(1, 0, 2).reshape(128, KC * Q)

    Wp = np.asarray(W_pack, dtype=np.float32)
    Wo = np.asarray(W_o, dtype=np.float32)
    kc_all = np.asarray(k_cache, dtype=np.float32)[0]   # [40, 2048, 128]
    vc_all = np.asarray(v_cache, dtype=np.float32)[0]
    mask = np.asarray(attention_mask, dtype=np.float32)
    mrows = mask[:, pos, :]                              # [40, 8, 2048]

    scale = np.float32(1.0 / math.sqrt(HD))

    in_maps = []
    for c in range(NCORES):
        r0 = c * HPC * HD
        r1 = (c + 1) * HPC * HD
        wsh = np.concatenate(
            [Wp[r0:r1] * scale, Wp[HIDDEN + r0:HIDDEN + r1],
             Wp[2 * HIDDEN + r0:2 * HIDDEN + r1]], axis=0)   # [1920, 5120]
        # [128 p, 40 kc, 1920 m]
        wpT = np.ascontiguousarray(
            wsh.T.reshape(KC, 128, MQKV).transpose(1, 0, 2)).astype(np.float16)
        heads = slice(c * HPC, (c + 1) * HPC)
        # [128 d, 5 h, 2048 pos]  (old cache as-is: winner columns are
        # neutralized via the -1e30 mask, not by insertion)
        kcT = np.ascontiguousarray(
            kc_all[heads].transpose(2, 0, 1)).astype(np.float16)
        # [128 p, 5 h, 16 c, 128 d]
        vcc = np.ascontiguousarray(
            vc_all[heads].reshape(HPC, NPOS, 128, HD)
            .transpose(2, 0, 1, 3)).astype(np.float16)
        # constants blob
        cb = np.zeros((128, CB_N), dtype=np.float32)
        # maskT[p, h, cj, t] = mrows[own_h, t, cj*128+p]; winner columns
        # -> -1e30 (their exp terms come from the corr side path instead)
        mT = mrows[heads].reshape(HPC, Q, NPOS, 128).transpose(3, 0, 2, 1).copy()
        for t in range(Q):
            if winners[t]:
                mT[pos[t] % 128, :, pos[t] // 128, :] = np.float32(-1e30)
        cb[:, CB_MASKT:CB_MASKT + HPC * NPOS * Q] = mT.reshape(128, -1)
        cb[0:Q, CB_EYE8:CB_EYE8 + Q] = np.eye(Q, dtype=np.float32)
        # maskN[j, h, t] = mrows[own_h, t, pos_j]; -1e30 for duplicate losers
        mN = mrows[heads][:, :, pos].transpose(2, 0, 1).copy()
        for j in range(Q):
            if not winners[j]:
                mN[j] = np.float32(-1e30)
        cb[0:Q, CB_MASKN:CB_MASKN + HPC * Q] = mN.reshape(Q, -1)
        cb[0:1, CB_ONES:CB_ONES + 128] = 1.0
        cb[0:Q, CB_ONES8:CB_ONES8 + 1] = 1.0
        # [128 p, 10 piece, 5 h, 512 n]
        wo = np.ascontiguousarray(
            Wo[:, r0:r1].reshape(NPIECE, WO_P, HPC, 128)
            .transpose(3, 0, 2, 1)).astype(np.float16)
        in_maps.append({
            "hsT": hsT, "wpT": wpT, "kcT": kcT, "vc": vcc, "cb": cb, "wo": wo,
        })
    return in_maps, pos, winners


def kernel(input_pos, hidden_states, attention_mask, W_pack, W_o,
           k_cache, v_cache, _profile=False):
    in_maps, pos, winners = _prep_inputs(
        input_pos, hidden_states, attention_mask, W_pack, W_o, k_cache, v_cache)
    nc = _get_program(pos, winners)
    res = run_bass_kernel_spmd(nc, in_maps, list(range(NCORES)), trace=_profile)
    out = np.zeros((Q, HIDDEN), dtype=np.float64)
    for r in res.results:
        out += r["out"].astype(np.float64)
    full = out.astype(np.float32).reshape(1, Q, HIDDEN)
    if _profile:
        return full, res
    return full


# revision 12
# speedup vs baseline: 1.0694x; 1.0694x over previous
"""Baichuan attention decode step on 8 Trainium2 NeuronCores (Bass/Tile).

Head-sharded tensor parallel: 40 heads -> 5 heads per core.
Per core:
  - QKV projection for own heads:  qkv[8,1920] = hs[8,5120] @ Wsh.T
    (Wsh = own-head rows of W_pack, q-rows pre-scaled by 1/sqrt(128))
  - scores (transposed): sT[pos,tok] per head via PE over the OLD k cache,
    + mask, exp on ACT. Winner columns (cache slots being overwritten)
    carry -1e30 in the shipped mask, so their stale-k exp terms vanish.
  - the 8 new tokens' scores come from a rank-8 side path:
    corr[j,h,t] = exp(k_new_j . q_t + mask[j,t]) (losers of duplicate
    positions get -1e30 -> 0). corr feeds both the denominator (ones8
    matmul) and the attention numerator (x v_new).
  - softmax denominator via ones-matmul + strided DVE reduce; division
    applied to the attention output (scale-after-matmul)
  - o_proj partial: out[8,5120] = attn[8,640] @ W_o[:,own_cols].T
Host sums the 8 partial outputs (the "all-reduce").

Only the 8 mask rows at input_pos are shipped to the device (gathered on
host), and input_pos is baked into the compiled program (recompiled per
distinct input_pos, cached).

All HBM-resident streams (W_pack, W_o, k/v caches, hidden states) are
fp16; matmuls accumulate in fp32 PSUM; softmax bookkeeping stays fp32.

DMA plan (the kernel is DMA-bound; HWDGE descriptor generation costs
~0.7-1.3us per dma_start, serial per ring, so starts are batched):
  sync ring, in strict consumption order: W_pack in groups (bufs=3
    prefetch), k cache (1 start), v cache (1 start), W_o in 10 pieces
    (SBUF-resident; o_proj rides them piece by piece, so only the last
    piece's matmul+store chain is on the critical tail).
  scalar ring: only tiny/late data — hsT(+ones column) lands during
    the sync ring's preamble, the packed constants blob
    (maskT/maskN/eye8/ones/ones8), and the paired output stores.
    Anything sizable on this ring gets starved behind the sync stream
    (measured), so the bulk stream stays on one ring.
"""

import os
import sys
import math
from contextlib import ExitStack

import numpy as np

for _p in ("/opt/trn_rl_repo", "/opt/trn_rl_repo/concourse"):
    if os.path.isdir(_p) and _p not in sys.path:
        sys.path.insert(0, _p)

import concourse.tile as tile  # noqa: E402
from concourse import bacc, mybir  # noqa: E402
from concourse.bass_utils import run_bass_kernel_spmd  # noqa: E402

F32 = mybir.dt.float32
F16 = mybir.dt.float16

HIDDEN = 5120
NH = 40
HD = 128
L = 2048
Q = 8
NCORES = 8
HPC = NH // NCORES          # 5 heads per core
KC = HIDDEN // 128          # 40 contraction chunks
MQKV = 3 * HPC * HD         # 1920 qkv output dim per core
NPOS = L // 128             # 16 position chunks
WP_G = 4                    # wpT k-chunks per DMA group
WO_P = 512                  # o_proj N-piece size (1 PSUM bank)
NPIECE = HIDDEN // WO_P     # 10

# constants blob layout (fp32, [128, CB_N]):
CB_MASKT = 0                          # [128, 640] maskT
CB_EYE8 = CB_MASKT + HPC * NPOS * Q   # [0:8, 640:648] eye8
CB_MASKN = CB_EYE8 + Q                # [0:8, 648:688] maskN
CB_ONES = CB_MASKN + HPC * Q          # [0:1, 688:816] ones row
CB_ONES8 = CB_ONES + 128              # [0:8, 816:817] ones8 col
CB_N = CB_ONES8 + 1

_CACHE = {}


def _build_program(pos, winners):
    """Build the SPMD Bass program with input_pos baked in.

    pos: list of 8 ints. winners: list of bools (True if token t's cache
    write survives, i.e. it is the last occurrence of that position).
    """
    nc = bacc.Bacc("TRN2", target_bir_lowering=False, debug=False)

    hsT_d = nc.dram_tensor("hsT", [128, KC * Q + 1], F16, kind="ExternalInput")
    wpT_d = nc.dram_tensor("wpT", [128, KC, MQKV], F16, kind="ExternalInput")
    kcT_d = nc.dram_tensor("kcT", [128, HPC, L], F16, kind="ExternalInput")
    vc_d = nc.dram_tensor("vc", [128, HPC, NPOS, HD], F16, kind="ExternalInput")
    cb_d = nc.dram_tensor("cb", [128, CB_N], F32, kind="ExternalInput")
    wo_d = nc.dram_tensor("wo", [128, NPIECE, HPC, WO_P], F16, kind="ExternalInput")
    out_d = nc.dram_tensor("out", [Q, HIDDEN], F32, kind="ExternalOutput")

    with tile.TileContext(nc) as tc, ExitStack() as ctx:
        sb = ctx.enter_context(tc.tile_pool(name="sb", bufs=1))
        ps = ctx.enter_context(tc.tile_pool(name="ps", bufs=1, space="PSUM"))

        # ---- big stream all on the sync ring in strict consumption order
        # (a second ring's transfers get starved behind this one's queued
        # descriptors, so only tiny/late data rides the scalar ring: hsT
        # lands in the preamble window before the sync stream saturates) ----
        hsTt = sb.tile([128, KC * Q + 1], F16, tag="hsT")
        nc.scalar.dma_start(hsTt[:], hsT_d.ap())
        hsT = hsTt[:, 0:KC * Q].rearrange("p (k t) -> p k t", k=KC)
        ones_r = hsTt[:, KC * Q:KC * Q + 1]           # fp16 ones column

        # W_pack chunk 0 also rides the scalar ring: it transfers during
        # the sync ring's descriptor-gen window, so engines never idle at
        # the start. Chunks 1+ stream on sync in consumption order.
        groups = [2, 2] + [WP_G] * ((KC - 4) // WP_G)
        assert sum(groups) == KC
        wp_tiles = []
        g0 = 0
        for gi, gn in enumerate(groups):
            wp = sb.tile([128, WP_G, MQKV], F16, tag="wstream", bufs=3)
            nc.sync.dma_start(wp[:, 0:gn, :], wpT_d.ap()[:, g0:g0 + gn, :])
            wp_tiles.append((g0, gn, wp))
            g0 += gn

        # constants blob on the scalar ring (needed mid-kernel; off the
        # critical stream)
        cb = sb.tile([128, CB_N], F32, tag="cb")
        nc.scalar.dma_start(cb[:], cb_d.ap())
        maskT = cb[:, CB_MASKT:CB_MASKT + HPC * NPOS * Q].rearrange(
            "p (h c t) -> p h c t", h=HPC, c=NPOS)
        eye8 = cb[0:Q, CB_EYE8:CB_EYE8 + Q]
        maskN = cb[0:Q, CB_MASKN:CB_MASKN + HPC * Q].rearrange(
            "p (h t) -> p h t", h=HPC)
        ones_row = cb[0:1, CB_ONES:CB_ONES + 128]
        ones8 = cb[0:Q, CB_ONES8:CB_ONES8 + 1]

        # ---- big stream on the sync ring in strict consumption order ----
        kcT = sb.tile([128, HPC, L], F16, tag="kcT")
        nc.sync.dma_start(kcT[:], kcT_d.ap())
        vc = sb.tile([128, HPC, NPOS, HD], F16, tag="vc")
        nc.sync.dma_start(vc[:], vc_d.ap())
        # W_o fully resident: its stream overlaps the scores/attention
        # phase and o_proj consumes it piece by piece as it lands.
        wo_sb = sb.tile([128, NPIECE, HPC, WO_P], F16, tag="wo")
        for piece in range(NPIECE):
            nc.sync.dma_start(wo_sb[:, piece], wo_d.ap()[:, piece, :, :])

        # ---- QKV projection: qkv[8, 1920] ----
        ps_qkv = ps.tile([Q, MQKV], F32, tag="A")
        nslices = [(0, 512), (512, 1024), (1024, 1536), (1536, MQKV)]
        for (g0, gn, wp) in wp_tiles:
            for i in range(gn):
                kc = g0 + i
                for (n0, n1) in nslices:
                    nc.tensor.matmul(
                        ps_qkv[0:Q, n0:n1],
                        hsT[:, kc, :],
                        wp[:, i, n0:n1],
                        start=(kc == 0),
                        stop=(kc == KC - 1),
                    )
        # q block first: it alone gates the scores matmuls
        qkv = sb.tile([Q, MQKV], F32, tag="qkvsb")
        nc.vector.tensor_copy(qkv[0:Q, 0:HPC * HD], ps_qkv[0:Q, 0:HPC * HD])

        # ---- transposes: qT per head, then scores ----
        qT = sb.tile([128, HPC, Q], F16, tag="qT")
        tq = ps.tile([128, HPC, Q], F32, tag="S1", bufs=2)
        for h in range(HPC):
            nc.tensor.transpose(tq[:, h, :], qkv[0:Q, h * HD:(h + 1) * HD], eye8)
        nc.vector.tensor_copy(qT[:], tq[:])

        # scores (transposed) over the OLD cache; winner columns are
        # masked to -1e30 host-side so their stale-k terms exp to 0.
        ps_sc = ps.tile([128, HPC, NPOS, Q], F32, tag="A")
        for h in range(HPC):
            for cj in range(NPOS):
                nc.tensor.matmul(
                    ps_sc[:, h, cj, :],
                    kcT[:, h, cj * 128:(cj + 1) * 128],
                    qT[:, h, :],
                    start=True,
                    stop=True,
                )

        # k/v blocks of qkv; kT_new transposes (PE order: after scores)
        nc.vector.tensor_copy(qkv[0:Q, HPC * HD:], ps_qkv[0:Q, HPC * HD:])
        tk = ps.tile([128, HPC, Q], F32, tag="S1", bufs=2)
        for h in range(HPC):
            nc.tensor.transpose(
                tk[:, h, :], qkv[0:Q, HPC * HD + h * HD:HPC * HD + (h + 1) * HD], eye8
            )

        # ---- + mask, exp (batched over heads) ----
        scT = sb.tile([128, HPC, NPOS, Q], F32, tag="scT")
        nc.vector.tensor_add(scT[:], ps_sc[:], maskT)
        expT = sb.tile([128, HPC, NPOS, Q], F16, tag="expT")
        nc.scalar.activation(expT[:], scT[:], mybir.ActivationFunctionType.Exp)

        # ---- corr = exp(k_new . q + maskN) (rank-8 side path) ----
        ktn = sb.tile([128, HPC, Q], F16, tag="ktn")
        nc.vector.tensor_copy(ktn[:], tk[:])
        corr_ps = ps.tile([Q, HPC, Q], F32, tag="S1", bufs=2)
        for h in range(HPC):
            nc.tensor.matmul(
                corr_ps[0:Q, h, :], ktn[:, h, :], qT[:, h, :], start=True, stop=True
            )
        corr_s = sb.tile([Q, HPC, Q], F32, tag="corrs")
        nc.vector.tensor_add(corr_s[:], corr_ps[:], maskN)
        corr = sb.tile([Q, HPC, Q], F32, tag="corr")
        nc.scalar.activation(corr[:], corr_s[:], mybir.ActivationFunctionType.Exp)

        # ---- denominators ----
        sums = sb.tile([1, HPC, Q], F32, tag="sums")
        for h in range(HPC):
            ps_sum = ps.tile([1, NPOS, Q], F32, tag="S1", bufs=2)
            nc.tensor.matmul(
                ps_sum[0:1],
                ones_r,
                expT[:, h].rearrange("p c t -> p (c t)"),
                start=True,
                stop=True,
            )
            nc.vector.tensor_reduce(
                sums[0:1, h, :],
                ps_sum.rearrange("p c t -> p t c"),
                axis=mybir.AxisListType.X,
                op=mybir.AluOpType.add,
            )
        # + the new tokens' exp terms
        ps_cs = ps.tile([1, HPC, Q], F32, tag="S1", bufs=2)
        nc.tensor.matmul(
            ps_cs[0:1],
            ones8,
            corr.rearrange("j h t -> j (h t)"),
            start=True,
            stop=True,
        )
        sums_f = sb.tile([1, HPC, Q], F32, tag="sumsf")
        nc.vector.tensor_add(sums_f[:], sums[:], ps_cs[:])

        # ---- attention numerator ----
        ps_at = ps.tile([128, HPC, Q], F32, tag="S1", bufs=2)
        for h in range(HPC):
            for cj in range(NPOS):
                nc.tensor.matmul(
                    ps_at[:, h, :],
                    vc[:, h, cj, :],
                    expT[:, h, cj, :],
                    start=(cj == 0),
                    stop=False,
                )
            # + rank-8 correction with the new v rows (v_new = qkv v-slice)
            nc.tensor.matmul(
                ps_at[:, h, :],
                qkv[0:Q, 2 * HPC * HD + h * HD:2 * HPC * HD + (h + 1) * HD],
                corr[:, h, :],
                start=False,
                stop=True,
            )

        # ---- reciprocals -> broadcast over partitions ----
        recip = sb.tile([1, HPC * Q], F32, tag="recip")
        nc.vector.reciprocal(recip[:], sums_f.rearrange("p h t -> p (h t)"))
        ps_bc = ps.tile([128, HPC * Q], F32, tag="S1", bufs=2)
        nc.tensor.matmul(ps_bc[:], ones_row, recip[0:1, :], start=True, stop=True)
        bc = sb.tile([128, HPC * Q], F32, tag="bcsb")
        nc.vector.tensor_copy(bc[:], ps_bc[:])
        attn = sb.tile([128, HPC * Q], F16, tag="attn")
        nc.vector.tensor_mul(attn[:], ps_at.rearrange("p h t -> p (h t)"), bc[:])

        # ---- o_proj partial: out[8, 5120], shipped out per piece on the
        # scalar ring so only the last piece's store is on the tail ----
        out_sb = sb.tile([Q, HIDDEN], F32, tag="outsb")
        for piece in range(NPIECE):
            ps_o = ps.tile([Q, WO_P], F32, tag="PO", bufs=2)
            for h in range(HPC):
                nc.tensor.matmul(
                    ps_o[0:Q, :],
                    attn[:, h * Q:(h + 1) * Q],
                    wo_sb[:, piece, h, :],
                    start=(h == 0),
                    stop=(h == HPC - 1),
                )
            n0 = piece * WO_P
            nc.vector.tensor_copy(out_sb[0:Q, n0:n0 + WO_P], ps_o[:])
            if piece % 2 == 1:
                m0 = (piece - 1) * WO_P
                nc.scalar.dma_start(
                    out_d.ap()[:, m0:m0 + 2 * WO_P], out_sb[0:Q, m0:m0 + 2 * WO_P])

    nc.compile()
    return nc


def _get_program(pos, winners):
    key = (tuple(pos), tuple(winners))
    if key not in _CACHE:
        _CACHE[key] = _build_program(pos, winners)
    return _CACHE[key]


def _prep_inputs(input_pos, hidden_states, attention_mask, W_pack, W_o,
                 k_cache, v_cache):
    """Host-side sharding: returns (in_maps, pos, winners)."""
    pos = [int(p) for p in np.asarray(input_pos).reshape(-1)]
    last = {}
    for t, p in enumerate(pos):
        last[p] = t
    winners = [last[p] == t for t, p in enumerate(pos)]

    hs = np.asarray(hidden_states, dtype=np.float32).reshape(Q, HIDDEN)
    # hsT[p, kc*8+t] = hs[t, kc*128+p]; final column = fp16 ones
    hsT = np.ones((128, KC * Q + 1), dtype=np.float16)
    hsT[:, 0:KC * Q] = hs.T.reshape(KC, 128, Q).transpose(1, 0, 2).reshape(128, KC * Q)

    Wp = np.asarray(W_pack, dtype=np.float32)
    Wo = np.asarray(W_o, dtype=np.float32)
    kc_all = np.asarray(k_cache, dtype=np.float32)[0]   # [40, 2048, 128]
    vc_all = np.asarray(v_cache, dtype=np.float32)[0]
    mask = np.asarray(attention_mask, dtype=np.float32)
    mrows = mask[:, pos, :]                              # [40, 8, 2048]

    scale = np.float32(1.0 / math.sqrt(HD))

    in_maps = []
    for c in range(NCORES):
        r0 = c * HPC * HD
        r1 = (c + 1) * HPC * HD
        wsh = np.concatenate(
            [Wp[r0:r1] * scale, Wp[HIDDEN + r0:HIDDEN + r1],
             Wp[2 * HIDDEN + r0:2 * HIDDEN + r1]], axis=0)   # [1920, 5120]
        # [128 p, 40 kc, 1920 m]
        wpT = np.ascontiguousarray(
            wsh.T.reshape(KC, 128, MQKV).transpose(1, 0, 2)).astype(np.float16)
        heads = slice(c * HPC, (c + 1) * HPC)
        # [128 d, 5 h, 2048 pos]  (old cache as-is: winner columns are
        # neutralized via the -1e30 mask, not by insertion)
        kcT = np.ascontiguousarray(
            kc_all[heads].transpose(2, 0, 1)).astype(np.float16)
        # [128 p, 5 h, 16 c, 128 d]
        vcc = np.ascontiguousarray(
            vc_all[heads].reshape(HPC, NPOS, 128, HD)
            .transpose(2, 0, 1, 3)).astype(np.float16)
        # constants blob
        cb = np.zeros((128, CB_N), dtype=np.float32)
        # maskT[p, h, cj, t] = mrows[own_h, t, cj*128+p]; winner columns
        # -> -1e30 (their exp terms come from the corr side path instead)
        mT = mrows[heads].reshape(HPC, Q, NPOS, 128).transpose(3, 0, 2, 1).copy()
        for t in range(Q):
            if winners[t]:
                mT[pos[t] % 128, :, pos[t] // 128, :] = np.float32(-1e30)
        cb[:, CB_MASKT:CB_MASKT + HPC * NPOS * Q] = mT.reshape(128, -1)
        cb[0:Q, CB_EYE8:CB_EYE8 + Q] = np.eye(Q, dtype=np.float32)
        # maskN[j, h, t] = mrows[own_h, t, pos_j]; -1e30 for duplicate losers
        mN = mrows[heads][:, :, pos].transpose(2, 0, 1).copy()
        for j in range(Q):
            if not winners[j]:
                mN[j] = np.float32(-1e30)
        cb[0:Q, CB_MASKN:CB_MASKN + HPC * Q] = mN.reshape(Q, -1)
        cb[0:1, CB_ONES:CB_ONES + 128] = 1.0
        cb[0:Q, CB_ONES8:CB_ONES8 + 1] = 1.0
        # [128 p, 10 piece, 5 h, 512 n]
        wo = np.ascontiguousarray(
            Wo[:, r0:r1].reshape(NPIECE, WO_P, HPC, 128)
            .transpose(3, 0, 2, 1)).astype(np.float16)
        in_maps.append({
            "hsT": hsT, "wpT": wpT, "kcT": kcT, "vc": vcc, "cb": cb, "wo": wo,
        })
    return in_maps, pos, winners


def kernel(input_pos, hidden_states, attention_mask, W_pack, W_o,
           k_cache, v_cache, _profile=False):
    in_maps, pos, winners = _prep_inputs(
        input_pos, hidden_states, attention_mask, W_pack, W_o, k_cache, v_cache)
    nc = _get_program(pos, winners)
    res = run_bass_kernel_spmd(nc, in_maps, list(range(NCORES)), trace=_profile)
    out = np.zeros((Q, HIDDEN), dtype=np.float64)
    for r in res.results:
        out += r["out"].astype(np.float64)
    full = out.astype(np.float32).reshape(1, Q, HIDDEN)
    if _profile:
        return full, res
    return full


# revision 14
# speedup vs baseline: 1.0878x; 1.0172x over previous
"""Baichuan attention decode step on 8 Trainium2 NeuronCores (Bass/Tile).

Head-sharded tensor parallel: 40 heads -> 5 heads per core.
Per core:
  - QKV projection for own heads:  qkv[8,1920] = hs[8,5120] @ Wsh.T
    (Wsh = own-head rows of W_pack, q-rows pre-scaled by 1/sqrt(128))
  - scores (transposed): sT[pos,tok] per head via PE over the OLD k cache,
    + mask, exp on ACT. Winner columns (cache slots being overwritten)
    carry -1e30 in the shipped mask, so their stale-k exp terms vanish.
  - the 8 new tokens' scores come from a rank-8 side path:
    corr[j,h,t] = exp(k_new_j . q_t + mask[j,t]) (losers of duplicate
    positions get -1e30 -> 0). corr feeds both the denominator (ones8
    matmul) and the attention numerator (x v_new).
  - softmax denominator via ones-matmul + strided DVE reduce; division
    applied to the attention output (scale-after-matmul)
  - o_proj partial: out[8,5120] = attn[8,640] @ W_o[:,own_cols].T
Host sums the 8 partial outputs (the "all-reduce").

Only the 8 mask rows at input_pos are shipped to the device (gathered on
host), and input_pos is baked into the compiled program (recompiled per
distinct input_pos, cached).

All HBM-resident streams (W_pack, W_o, k/v caches, hidden states, mask
rows) are fp16; matmuls accumulate in fp32 PSUM; softmax bookkeeping
(exp input, denominators, reciprocals) stays fp32. Masked-out columns
use -60000 (fp16-representable; exp underflows to 0 in fp32).

DMA plan (the kernel is DMA-bound; HWDGE descriptor generation costs
~0.7-1.3us per dma_start, serial per ring, so starts are batched):
  sync ring, in strict consumption order: W_pack in groups (bufs=3
    prefetch), k cache (1 start), v cache (1 start), W_o in 9 pieces
    + 2 half pieces (SBUF-resident; o_proj rides them piece by piece,
    and the half-size final piece shortens the post-stream tail chain).
  scalar ring: only tiny/late data — hsT(+ones column) lands during
    the sync ring's preamble, the packed constants blob
    (maskT/maskN/eye8/ones/ones8), and the paired output stores.
    Anything sizable on this ring gets starved behind the sync stream
    (measured), so the bulk stream stays on one ring.
"""

import os
import sys
import math
from contextlib import ExitStack

import numpy as np

for _p in ("/opt/trn_rl_repo", "/opt/trn_rl_repo/concourse"):
    if os.path.isdir(_p) and _p not in sys.path:
        sys.path.insert(0, _p)

import concourse.tile as tile  # noqa: E402
from concourse import bacc, mybir  # noqa: E402
from concourse.bass_utils import run_bass_kernel_spmd  # noqa: E402

F32 = mybir.dt.float32
F16 = mybir.dt.float16

HIDDEN = 5120
NH = 40
HD = 128
L = 2048
Q = 8
NCORES = 8
HPC = NH // NCORES          # 5 heads per core
KC = HIDDEN // 128          # 40 contraction chunks
MQKV = 3 * HPC * HD         # 1920 qkv output dim per core
NPOS = L // 128             # 16 position chunks
WP_G = 4                    # wpT k-chunks per DMA group
WO_P = 512                  # o_proj N-piece size (1 PSUM bank)
NPIECE = HIDDEN // WO_P     # 10

# constants blob layout (fp32, [128, CB_N]); maskT ships separately fp16
CB_EYE8 = 0                           # [0:8, 0:8] eye8
CB_MASKN = CB_EYE8 + Q                # [0:8, 8:48] maskN
CB_ONES = CB_MASKN + HPC * Q          # [0:1, 48:176] ones row
CB_ONES8 = CB_ONES + 128              # [0:8, 176:177] ones8 col
CB_N = CB_ONES8 + 1
MASK_NEG = -60000.0                   # fp16-representable; exp() -> 0 in fp32

_CACHE = {}


def _build_program(pos, winners):
    """Build the SPMD Bass program with input_pos baked in.

    pos: list of 8 ints. winners: list of bools (True if token t's cache
    write survives, i.e. it is the last occurrence of that position).
    """
    nc = bacc.Bacc("TRN2", target_bir_lowering=False, debug=False)

    hsT_d = nc.dram_tensor("hsT", [128, KC * Q + 1], F16, kind="ExternalInput")
    wpT_d = nc.dram_tensor("wpT", [128, KC, MQKV], F16, kind="ExternalInput")
    kcT_d = nc.dram_tensor("kcT", [128, HPC, L], F16, kind="ExternalInput")
    vc_d = nc.dram_tensor("vc", [128, HPC, NPOS, HD], F16, kind="ExternalInput")
    cb_d = nc.dram_tensor("cb", [128, CB_N], F32, kind="ExternalInput")
    mkT_d = nc.dram_tensor("mkT", [128, HPC * NPOS * Q], F16, kind="ExternalInput")
    wo_d = nc.dram_tensor("wo", [128, NPIECE, HPC, WO_P], F16, kind="ExternalInput")
    out_d = nc.dram_tensor("out", [Q, HIDDEN], F32, kind="ExternalOutput")

    with tile.TileContext(nc) as tc, ExitStack() as ctx:
        sb = ctx.enter_context(tc.tile_pool(name="sb", bufs=1))
        ps = ctx.enter_context(tc.tile_pool(name="ps", bufs=1, space="PSUM"))

        # ---- big stream all on the sync ring in strict consumption order
        # (a second ring's transfers get starved behind this one's queued
        # descriptors, so only tiny/late data rides the scalar ring: hsT
        # lands in the preamble window before the sync stream saturates) ----
        hsTt = sb.tile([128, KC * Q + 1], F16, tag="hsT")
        nc.scalar.dma_start(hsTt[:], hsT_d.ap())
        hsT = hsTt[:, 0:KC * Q].rearrange("p (k t) -> p k t", k=KC)
        ones_r = hsTt[:, KC * Q:KC * Q + 1]           # fp16 ones column

        groups = [2, 2] + [WP_G] * ((KC - 4) // WP_G)
        assert sum(groups) == KC
        wp_tiles = []
        g0 = 0
        for gi, gn in enumerate(groups):
            wp = sb.tile([128, WP_G, MQKV], F16, tag="wstream", bufs=3)
            nc.sync.dma_start(wp[:, 0:gn, :], wpT_d.ap()[:, g0:g0 + gn, :])
            wp_tiles.append((g0, gn, wp))
            g0 += gn

        # constants blob on the scalar ring (needed mid-kernel; off the
        # critical stream)
        cb = sb.tile([128, CB_N], F32, tag="cb")
        nc.scalar.dma_start(cb[:], cb_d.ap())
        mkT = sb.tile([128, HPC * NPOS * Q], F16, tag="mkT")
        nc.scalar.dma_start(mkT[:], mkT_d.ap())
        maskT = mkT.rearrange("p (h c t) -> p h c t", h=HPC, c=NPOS)
        eye8 = cb[0:Q, CB_EYE8:CB_EYE8 + Q]
        maskN = cb[0:Q, CB_MASKN:CB_MASKN + HPC * Q].rearrange(
            "p (h t) -> p h t", h=HPC)
        ones_row = cb[0:1, CB_ONES:CB_ONES + 128]
        ones8 = cb[0:Q, CB_ONES8:CB_ONES8 + 1]

        # ---- big stream on the sync ring in strict consumption order ----
        kcT = sb.tile([128, HPC, L], F16, tag="kcT")
        nc.sync.dma_start(kcT[:], kcT_d.ap())
        vc = sb.tile([128, HPC, NPOS, HD], F16, tag="vc")
        nc.sync.dma_start(vc[:], vc_d.ap())
        # W_o fully resident: its stream overlaps the scores/attention
        # phase and o_proj consumes it piece by piece as it lands.
        wo_sb = sb.tile([128, NPIECE, HPC, WO_P], F16, tag="wo")
        for piece in range(NPIECE - 1):
            nc.sync.dma_start(wo_sb[:, piece], wo_d.ap()[:, piece, :, :])
        # last piece in halves: the final arrival feeds a half-size matmul
        # chain, shortening the post-stream tail
        LP = NPIECE - 1
        nc.sync.dma_start(wo_sb[:, LP, :, 0:WO_P // 2],
                          wo_d.ap()[:, LP, :, 0:WO_P // 2])
        nc.sync.dma_start(wo_sb[:, LP, :, WO_P // 2:],
                          wo_d.ap()[:, LP, :, WO_P // 2:])

        # ---- QKV projection: qkv[8, 1920] ----
        ps_qkv = ps.tile([Q, MQKV], F32, tag="A")
        nslices = [(0, 512), (512, 1024), (1024, 1536), (1536, MQKV)]
        for (g0, gn, wp) in wp_tiles:
            for i in range(gn):
                kc = g0 + i
                for (n0, n1) in nslices:
                    nc.tensor.matmul(
                        ps_qkv[0:Q, n0:n1],
                        hsT[:, kc, :],
                        wp[:, i, n0:n1],
                        start=(kc == 0),
                        stop=(kc == KC - 1),
                    )
        # q block first: it alone gates the scores matmuls
        qkv = sb.tile([Q, MQKV], F32, tag="qkvsb")
        nc.vector.tensor_copy(qkv[0:Q, 0:HPC * HD], ps_qkv[0:Q, 0:HPC * HD])

        # ---- transposes: qT per head, then scores ----
        qT = sb.tile([128, HPC, Q], F16, tag="qT")
        tq = ps.tile([128, HPC, Q], F32, tag="S1", bufs=2)
        for h in range(HPC):
            nc.tensor.transpose(tq[:, h, :], qkv[0:Q, h * HD:(h + 1) * HD], eye8)
        nc.vector.tensor_copy(qT[:], tq[:])

        # scores (transposed) over the OLD cache; winner columns are
        # masked to -1e30 host-side so their stale-k terms exp to 0.
        ps_sc = ps.tile([128, HPC, NPOS, Q], F32, tag="A")
        for h in range(HPC):
            for cj in range(NPOS):
                nc.tensor.matmul(
                    ps_sc[:, h, cj, :],
                    kcT[:, h, cj * 128:(cj + 1) * 128],
                    qT[:, h, :],
                    start=True,
                    stop=True,
                )

        # k/v blocks of qkv; kT_new transposes (PE order: after scores)
        nc.vector.tensor_copy(qkv[0:Q, HPC * HD:], ps_qkv[0:Q, HPC * HD:])
        tk = ps.tile([128, HPC, Q], F32, tag="S1", bufs=2)
        for h in range(HPC):
            nc.tensor.transpose(
                tk[:, h, :], qkv[0:Q, HPC * HD + h * HD:HPC * HD + (h + 1) * HD], eye8
            )

        # ---- + mask, exp (batched over heads) ----
        scT = sb.tile([128, HPC, NPOS, Q], F32, tag="scT")
        nc.vector.tensor_add(scT[:], ps_sc[:], maskT)
        expT = sb.tile([128, HPC, NPOS, Q], F16, tag="expT")
        nc.scalar.activation(expT[:], scT[:], mybir.ActivationFunctionType.Exp)

        # ---- corr = exp(k_new . q + maskN) (rank-8 side path) ----
        ktn = sb.tile([128, HPC, Q], F16, tag="ktn")
        nc.vector.tensor_copy(ktn[:], tk[:])
        corr_ps = ps.tile([Q, HPC, Q], F32, tag="S1", bufs=2)
        for h in range(HPC):
            nc.tensor.matmul(
                corr_ps[0:Q, h, :], ktn[:, h, :], qT[:, h, :], start=True, stop=True
            )
        corr_s = sb.tile([Q, HPC, Q], F32, tag="corrs")
        nc.vector.tensor_add(corr_s[:], corr_ps[:], maskN)
        corr = sb.tile([Q, HPC, Q], F32, tag="corr")
        nc.scalar.activation(corr[:], corr_s[:], mybir.ActivationFunctionType.Exp)

        # ---- denominators ----
        sums = sb.tile([1, HPC, Q], F32, tag="sums")
        for h in range(HPC):
            ps_sum = ps.tile([1, NPOS, Q], F32, tag="S1", bufs=2)
            nc.tensor.matmul(
                ps_sum[0:1],
                ones_r,
                expT[:, h].rearrange("p c t -> p (c t)"),
                start=True,
                stop=True,
            )
            nc.vector.tensor_reduce(
                sums[0:1, h, :],
                ps_sum.rearrange("p c t -> p t c"),
                axis=mybir.AxisListType.X,
                op=mybir.AluOpType.add,
            )
        # + the new tokens' exp terms
        ps_cs = ps.tile([1, HPC, Q], F32, tag="S1", bufs=2)
        nc.tensor.matmul(
            ps_cs[0:1],
            ones8,
            corr.rearrange("j h t -> j (h t)"),
            start=True,
            stop=True,
        )
        sums_f = sb.tile([1, HPC, Q], F32, tag="sumsf")
        nc.vector.tensor_add(sums_f[:], sums[:], ps_cs[:])

        # ---- attention numerator ----
        ps_at = ps.tile([128, HPC, Q], F32, tag="S1", bufs=2)
        for h in range(HPC):
            for cj in range(NPOS):
                nc.tensor.matmul(
                    ps_at[:, h, :],
                    vc[:, h, cj, :],
                    expT[:, h, cj, :],
                    start=(cj == 0),
                    stop=False,
                )
            # + rank-8 correction with the new v rows (v_new = qkv v-slice)
            nc.tensor.matmul(
                ps_at[:, h, :],
                qkv[0:Q, 2 * HPC * HD + h * HD:2 * HPC * HD + (h + 1) * HD],
                corr[:, h, :],
                start=False,
                stop=True,
            )

        # ---- reciprocals -> broadcast over partitions ----
        recip = sb.tile([1, HPC * Q], F32, tag="recip")
        nc.vector.reciprocal(recip[:], sums_f.rearrange("p h t -> p (h t)"))
        ps_bc = ps.tile([128, HPC * Q], F32, tag="S1", bufs=2)
        nc.tensor.matmul(ps_bc[:], ones_row, recip[0:1, :], start=True, stop=True)
        bc = sb.tile([128, HPC * Q], F32, tag="bcsb")
        nc.vector.tensor_copy(bc[:], ps_bc[:])
        attn = sb.tile([128, HPC * Q], F16, tag="attn")
        nc.vector.tensor_mul(attn[:], ps_at.rearrange("p h t -> p (h t)"), bc[:])

        # ---- o_proj partial: out[8, 5120], shipped out per piece on the
        # scalar ring so only the last piece's store is on the tail ----
        out_sb = sb.tile([Q, HIDDEN], F32, tag="outsb")
        H2 = WO_P // 2
        tasks = [(p * WO_P, WO_P) for p in range(NPIECE - 1)]
        tasks += [((NPIECE - 1) * WO_P, H2), ((NPIECE - 1) * WO_P + H2, H2)]
        done = 0
        for (n0, w) in tasks:
            ps_o = ps.tile([Q, WO_P], F32, tag="PO", bufs=2)
            for h in range(HPC):
                nc.tensor.matmul(
                    ps_o[0:Q, 0:w],
                    attn[:, h * Q:(h + 1) * Q],
                    wo_sb[:, n0 // WO_P, h, n0 % WO_P:n0 % WO_P + w],
                    start=(h == 0),
                    stop=(h == HPC - 1),
                )
            nc.vector.tensor_copy(out_sb[0:Q, n0:n0 + w], ps_o[0:Q, 0:w])
            # ship accumulated columns once >= 1024 are ready (or at the end)
            if (n0 + w) - done >= 2 * WO_P or (n0 + w) == HIDDEN:
                nc.scalar.dma_start(
                    out_d.ap()[:, done:n0 + w], out_sb[0:Q, done:n0 + w])
                done = n0 + w

    nc.compile()
    return nc


def _get_program(pos, winners):
    key = (tuple(pos), tuple(winners))
    if key not in _CACHE:
        _CACHE[key] = _build_program(pos, winners)
    return _CACHE[key]


def _prep_inputs(input_pos, hidden_states, attention_mask, W_pack, W_o,
                 k_cache, v_cache):
    """Host-side sharding: returns (in_maps, pos, winners)."""
    pos = [int(p) for p in np.asarray(input_pos).reshape(-1)]
    last = {}
    for t, p in enumerate(pos):
        last[p] = t
    winners = [last[p] == t for t, p in enumerate(pos)]

    hs = np.asarray(hidden_states, dtype=np.float32).reshape(Q, HIDDEN)
    # hsT[p, kc*8+t] = hs[t, kc*128+p]; final column = fp16 ones
    hsT = np.ones((128, KC * Q + 1), dtype=np.float16)
    hsT[:, 0:KC * Q] = hs.T.reshape(KC, 128, Q).transpose(1, 0, 2).reshape(128, KC * Q)

    Wp = np.asarray(W_pack, dtype=np.float32)
    Wo = np.asarray(W_o, dtype=np.float32)
    kc_all = np.asarray(k_cache, dtype=np.float32)[0]   # [40, 2048, 128]
    vc_all = np.asarray(v_cache, dtype=np.float32)[0]
    mask = np.asarray(attention_mask, dtype=np.float32)
    mrows = mask[:, pos, :]                              # [40, 8, 2048]

    scale = np.float32(1.0 / math.sqrt(HD))

    in_maps = []
    for c in range(NCORES):
        r0 = c * HPC * HD
        r1 = (c + 1) * HPC * HD
        wsh = np.concatenate(
            [Wp[r0:r1] * scale, Wp[HIDDEN + r0:HIDDEN + r1],
             Wp[2 * HIDDEN + r0:2 * HIDDEN + r1]], axis=0)   # [1920, 5120]
        # [128 p, 40 kc, 1920 m]
        wpT = np.ascontiguousarray(
            wsh.T.reshape(KC, 128, MQKV).transpose(1, 0, 2)).astype(np.float16)
        heads = slice(c * HPC, (c + 1) * HPC)
        # [128 d, 5 h, 2048 pos]  (old cache as-is: winner columns are
        # neutralized via the -1e30 mask, not by insertion)
        kcT = np.ascontiguousarray(
            kc_all[heads].transpose(2, 0, 1)).astype(np.float16)
        # [128 p, 5 h, 16 c, 128 d]
        vcc = np.ascontiguousarray(
            vc_all[heads].reshape(HPC, NPOS, 128, HD)
            .transpose(2, 0, 1, 3)).astype(np.float16)
        # constants blob + fp16 maskT
        cb = np.zeros((128, CB_N), dtype=np.float32)
        # maskT[p, h, cj, t] = mrows[own_h, t, cj*128+p]; winner columns
        # -> -60000 (their exp terms come from the corr side path instead)
        mT = mrows[heads].reshape(HPC, Q, NPOS, 128).transpose(3, 0, 2, 1).copy()
        for t in range(Q):
            if winners[t]:
                mT[pos[t] % 128, :, pos[t] // 128, :] = np.float32(MASK_NEG)
        mkT = mT.reshape(128, -1).astype(np.float16)
        cb[0:Q, CB_EYE8:CB_EYE8 + Q] = np.eye(Q, dtype=np.float32)
        # maskN[j, h, t] = mrows[own_h, t, pos_j]; -1e30 for duplicate losers
        mN = mrows[heads][:, :, pos].transpose(2, 0, 1).copy()
        for j in range(Q):
            if not winners[j]:
                mN[j] = np.float32(-1e30)
        cb[0:Q, CB_MASKN:CB_MASKN + HPC * Q] = mN.reshape(Q, -1)
        cb[0:1, CB_ONES:CB_ONES + 128] = 1.0
        cb[0:Q, CB_ONES8:CB_ONES8 + 1] = 1.0
        # [128 p, 10 piece, 5 h, 512 n]
        wo = np.ascontiguousarray(
            Wo[:, r0:r1].reshape(NPIECE, WO_P, HPC, 128)
            .transpose(3, 0, 2, 1)).astype(np.float16)
        in_maps.append({
            "hsT": hsT, "wpT": wpT, "kcT": kcT, "vc": vcc, "cb": cb,
            "mkT": mkT, "wo": wo,
        })
    return in_maps, pos, winners


def kernel(input_pos, hidden_states, attention_mask, W_pack, W_o,
           k_cache, v_cache, _profile=False):
    in_maps, pos, winners = _prep_inputs(
        input_pos, hidden_states, attention_mask, W_pack, W_o, k_cache, v_cache)
    nc = _get_program(pos, winners)
    res = run_bass_kernel_spmd(nc, in_maps, list(range(NCORES)), trace=_profile)
    out = np.zeros((Q, HIDDEN), dtype=np.float64)
    for r in res.results:
        out += r["out"].astype(np.float64)
    full = out.astype(np.float32).reshape(1, Q, HIDDEN)
    if _profile:
        return full, res
    return full


# revision 15
# speedup vs baseline: 1.1767x; 1.0817x over previous
"""Baichuan attention decode step on 8 Trainium2 NeuronCores (Bass/Tile).

Head-sharded tensor parallel: 40 heads -> 5 heads per core.
Per core:
  - QKV projection for own heads:  qkv[8,1920] = hs[8,5120] @ Wsh.T
    (Wsh = own-head rows of W_pack, q-rows pre-scaled by 1/sqrt(128))
  - scores (transposed): sT[pos,tok] per head via PE over the OLD k cache,
    + mask, exp on ACT. Winner columns (cache slots being overwritten)
    carry -1e30 in the shipped mask, so their stale-k exp terms vanish.
  - the 8 new tokens' scores come from a rank-8 side path:
    corr[j,h,t] = exp(k_new_j . q_t + mask[j,t]) (losers of duplicate
    positions get -1e30 -> 0). corr feeds both the denominator (ones8
    matmul) and the attention numerator (x v_new).
  - softmax denominator via ones-matmul + strided DVE reduce; division
    applied to the attention output (scale-after-matmul)
  - o_proj partial: out[8,5120] = attn[8,640] @ W_o[:,own_cols].T
Host sums the 8 partial outputs (the "all-reduce").

Only the 8 mask rows at input_pos are shipped to the device (gathered on
host), and input_pos is baked into the compiled program (recompiled per
distinct input_pos, cached).

All HBM-resident streams (W_pack, W_o, k/v caches, hidden states, mask
rows) are fp16; matmuls accumulate in fp32 PSUM; softmax bookkeeping
(exp input, denominators, reciprocals) stays fp32. Masked-out columns
use -60000 (fp16-representable; exp underflows to 0 in fp32).

DMA plan (the kernel is DMA-bound; HWDGE descriptor generation costs
~0.7-1.3us per dma_start, serial per ring, so starts are batched):
  sync ring, in strict consumption order: W_pack in groups (bufs=3
    prefetch), k cache (1 start), v cache (1 start), W_o in 9 pieces
    + 2 half pieces (SBUF-resident; o_proj rides them piece by piece,
    and the half-size final piece shortens the post-stream tail chain).
  scalar ring: only tiny/late data — hsT(+ones column) lands during
    the sync ring's preamble, the packed constants blob
    (maskT/maskN/eye8/ones/ones8), and the paired output stores.
    Anything sizable on this ring gets starved behind the sync stream
    (measured), so the bulk stream stays on one ring.
"""

import os
import sys
import math
from contextlib import ExitStack

import numpy as np

for _p in ("/opt/trn_rl_repo", "/opt/trn_rl_repo/concourse"):
    if os.path.isdir(_p) and _p not in sys.path:
        sys.path.insert(0, _p)

import concourse.tile as tile  # noqa: E402
from concourse import bacc, mybir  # noqa: E402
from concourse.bass_utils import run_bass_kernel_spmd  # noqa: E402

F32 = mybir.dt.float32
F16 = mybir.dt.float16

HIDDEN = 5120
NH = 40
HD = 128
L = 2048
Q = 8
NCORES = 8
HPC = NH // NCORES          # 5 heads per core
KC = HIDDEN // 128          # 40 contraction chunks
MQKV = 3 * HPC * HD         # 1920 qkv output dim per core
NPOS = L // 128             # 16 position chunks
WP_G = 4                    # wpT k-chunks per DMA group
WO_P = 512                  # o_proj N-piece size (1 PSUM bank)
NPIECE = HIDDEN // WO_P     # 10

# constants blob layout (fp32, [128, CB_N]); maskT ships separately fp16
CB_EYE8 = 0                           # [0:8, 0:8] eye8
CB_MASKN = CB_EYE8 + Q                # [0:8, 8:48] maskN
CB_ONES = CB_MASKN + HPC * Q          # [0:1, 48:176] ones row
CB_ONES8 = CB_ONES + 128              # [0:8, 176:177] ones8 col
CB_N = CB_ONES8 + 1
MASK_NEG = -60000.0                   # fp16-representable; exp() -> 0 in fp32

_CACHE = {}


def _build_program(pos, winners):
    """Build the SPMD Bass program with input_pos baked in.

    pos: list of 8 ints. winners: list of bools (True if token t's cache
    write survives, i.e. it is the last occurrence of that position).
    """
    nc = bacc.Bacc("TRN2", target_bir_lowering=False, debug=False)

    hsT_d = nc.dram_tensor("hsT", [128, KC * Q + 1], F16, kind="ExternalInput")
    wpT_d = nc.dram_tensor("wpT", [128, KC, MQKV], F16, kind="ExternalInput")
    kcT_d = nc.dram_tensor("kcT", [128, HPC, L], F16, kind="ExternalInput")
    vc_d = nc.dram_tensor("vc", [128, HPC, NPOS, HD], F16, kind="ExternalInput")
    cb_d = nc.dram_tensor("cb", [128, CB_N], F32, kind="ExternalInput")
    mkT_d = nc.dram_tensor("mkT", [128, HPC * NPOS * Q], F16, kind="ExternalInput")
    wo_d = nc.dram_tensor("wo", [128, NPIECE, HPC, WO_P], F16, kind="ExternalInput")
    out_d = nc.dram_tensor("out", [Q, HIDDEN], F32, kind="ExternalOutput")

    with tile.TileContext(nc) as tc, ExitStack() as ctx:
        sb = ctx.enter_context(tc.tile_pool(name="sb", bufs=1))
        ps = ctx.enter_context(tc.tile_pool(name="ps", bufs=1, space="PSUM"))

        # ---- big stream all on the sync ring in strict consumption order
        # (a second ring's transfers get starved behind this one's queued
        # descriptors, so only tiny/late data rides the scalar ring: hsT
        # lands in the preamble window before the sync stream saturates) ----
        hsTt = sb.tile([128, KC * Q + 1], F16, tag="hsT")
        nc.scalar.dma_start(hsTt[:], hsT_d.ap())
        hsT = hsTt[:, 0:KC * Q].rearrange("p (k t) -> p k t", k=KC)
        ones_r = hsTt[:, KC * Q:KC * Q + 1]           # fp16 ones column

        groups = [2, 2] + [WP_G] * ((KC - 4) // WP_G)
        assert sum(groups) == KC
        wp_tiles = []
        g0 = 0
        for gi, gn in enumerate(groups):
            wp = sb.tile([128, WP_G, MQKV], F16, tag="wstream", bufs=3)
            nc.sync.dma_start(wp[:, 0:gn, :], wpT_d.ap()[:, g0:g0 + gn, :])
            wp_tiles.append((g0, gn, wp))
            g0 += gn

        # constants blob on the scalar ring (needed mid-kernel; off the
        # critical stream)
        cb = sb.tile([128, CB_N], F32, tag="cb")
        nc.scalar.dma_start(cb[:], cb_d.ap())
        mkT = sb.tile([128, HPC * NPOS * Q], F16, tag="mkT")
        nc.scalar.dma_start(mkT[:], mkT_d.ap())
        maskT = mkT.rearrange("p (h c t) -> p h c t", h=HPC, c=NPOS)
        eye8 = cb[0:Q, CB_EYE8:CB_EYE8 + Q]
        maskN = cb[0:Q, CB_MASKN:CB_MASKN + HPC * Q].rearrange(
            "p (h t) -> p h t", h=HPC)
        ones_row = cb[0:1, CB_ONES:CB_ONES + 128]
        ones8 = cb[0:Q, CB_ONES8:CB_ONES8 + 1]

        # ---- big stream on the sync ring in strict consumption order ----
        kcT = sb.tile([128, HPC, L], F16, tag="kcT")
        nc.sync.dma_start(kcT[:], kcT_d.ap())
        vc = sb.tile([128, HPC, NPOS, HD], F16, tag="vc")
        nc.sync.dma_start(vc[:], vc_d.ap())
        # W_o fully resident: its stream overlaps the scores/attention
        # phase and o_proj consumes it piece by piece as it lands.
        wo_sb = sb.tile([128, NPIECE, HPC, WO_P], F16, tag="wo")
        # ship W_o in 1024-col pairs (fewer descriptor gens than per-piece;
        # o_proj still consumes 512-col pieces), then piece 8 alone, then
        # the last piece in halves: the final arrival feeds a half-size
        # matmul chain, shortening the post-stream tail
        for pair in range(4):
            nc.sync.dma_start(wo_sb[:, 2 * pair:2 * pair + 2],
                              wo_d.ap()[:, 2 * pair:2 * pair + 2, :, :])
        nc.sync.dma_start(wo_sb[:, 8], wo_d.ap()[:, 8, :, :])
        LP = NPIECE - 1
        nc.sync.dma_start(wo_sb[:, LP, :, 0:WO_P // 2],
                          wo_d.ap()[:, LP, :, 0:WO_P // 2])
        nc.sync.dma_start(wo_sb[:, LP, :, WO_P // 2:],
                          wo_d.ap()[:, LP, :, WO_P // 2:])

        # ---- QKV projection: qkv[8, 1920] ----
        ps_qkv = ps.tile([Q, MQKV], F32, tag="A")
        nslices = [(0, 512), (512, 1024), (1024, 1536), (1536, MQKV)]
        for (g0, gn, wp) in wp_tiles:
            for i in range(gn):
                kc = g0 + i
                for (n0, n1) in nslices:
                    nc.tensor.matmul(
                        ps_qkv[0:Q, n0:n1],
                        hsT[:, kc, :],
                        wp[:, i, n0:n1],
                        start=(kc == 0),
                        stop=(kc == KC - 1),
                    )
        # q block first: it alone gates the scores matmuls
        qkv = sb.tile([Q, MQKV], F32, tag="qkvsb")
        nc.vector.tensor_copy(qkv[0:Q, 0:HPC * HD], ps_qkv[0:Q, 0:HPC * HD])

        # ---- transposes: qT per head, then scores ----
        qT = sb.tile([128, HPC, Q], F16, tag="qT")
        tq = ps.tile([128, HPC, Q], F32, tag="S1", bufs=2)
        for h in range(HPC):
            nc.tensor.transpose(tq[:, h, :], qkv[0:Q, h * HD:(h + 1) * HD], eye8)
        nc.vector.tensor_copy(qT[:], tq[:])

        # scores (transposed) over the OLD cache; winner columns are
        # masked to -1e30 host-side so their stale-k terms exp to 0.
        ps_sc = ps.tile([128, HPC, NPOS, Q], F32, tag="A")
        for h in range(HPC):
            for cj in range(NPOS):
                nc.tensor.matmul(
                    ps_sc[:, h, cj, :],
                    kcT[:, h, cj * 128:(cj + 1) * 128],
                    qT[:, h, :],
                    start=True,
                    stop=True,
                )

        # k/v blocks of qkv; kT_new transposes (PE order: after scores)
        nc.vector.tensor_copy(qkv[0:Q, HPC * HD:], ps_qkv[0:Q, HPC * HD:])
        tk = ps.tile([128, HPC, Q], F32, tag="S1", bufs=2)
        for h in range(HPC):
            nc.tensor.transpose(
                tk[:, h, :], qkv[0:Q, HPC * HD + h * HD:HPC * HD + (h + 1) * HD], eye8
            )

        # ---- + mask, exp (batched over heads) ----
        scT = sb.tile([128, HPC, NPOS, Q], F32, tag="scT")
        nc.vector.tensor_add(scT[:], ps_sc[:], maskT)
        expT = sb.tile([128, HPC, NPOS, Q], F16, tag="expT")
        nc.scalar.activation(expT[:], scT[:], mybir.ActivationFunctionType.Exp)

        # ---- corr = exp(k_new . q + maskN) (rank-8 side path) ----
        ktn = sb.tile([128, HPC, Q], F16, tag="ktn")
        nc.vector.tensor_copy(ktn[:], tk[:])
        corr_ps = ps.tile([Q, HPC, Q], F32, tag="S1", bufs=2)
        for h in range(HPC):
            nc.tensor.matmul(
                corr_ps[0:Q, h, :], ktn[:, h, :], qT[:, h, :], start=True, stop=True
            )
        corr_s = sb.tile([Q, HPC, Q], F32, tag="corrs")
        nc.vector.tensor_add(corr_s[:], corr_ps[:], maskN)
        corr = sb.tile([Q, HPC, Q], F32, tag="corr")
        nc.scalar.activation(corr[:], corr_s[:], mybir.ActivationFunctionType.Exp)

        # ---- denominators ----
        sums = sb.tile([1, HPC, Q], F32, tag="sums")
        for h in range(HPC):
            ps_sum = ps.tile([1, NPOS, Q], F32, tag="S1", bufs=2)
            nc.tensor.matmul(
                ps_sum[0:1],
                ones_r,
                expT[:, h].rearrange("p c t -> p (c t)"),
                start=True,
                stop=True,
            )
            nc.vector.tensor_reduce(
                sums[0:1, h, :],
                ps_sum.rearrange("p c t -> p t c"),
                axis=mybir.AxisListType.X,
                op=mybir.AluOpType.add,
            )
        # + the new tokens' exp terms
        ps_cs = ps.tile([1, HPC, Q], F32, tag="S1", bufs=2)
        nc.tensor.matmul(
            ps_cs[0:1],
            ones8,
            corr.rearrange("j h t -> j (h t)"),
            start=True,
            stop=True,
        )
        sums_f = sb.tile([1, HPC, Q], F32, tag="sumsf")
        nc.vector.tensor_add(sums_f[:], sums[:], ps_cs[:])

        # ---- attention numerator ----
        ps_at = ps.tile([128, HPC, Q], F32, tag="S1", bufs=2)
        for h in range(HPC):
            for cj in range(NPOS):
                nc.tensor.matmul(
                    ps_at[:, h, :],
                    vc[:, h, cj, :],
                    expT[:, h, cj, :],
                    start=(cj == 0),
                    stop=False,
                )
            # + rank-8 correction with the new v rows (v_new = qkv v-slice)
            nc.tensor.matmul(
                ps_at[:, h, :],
                qkv[0:Q, 2 * HPC * HD + h * HD:2 * HPC * HD + (h + 1) * HD],
                corr[:, h, :],
                start=False,
                stop=True,
            )

        # ---- reciprocals -> broadcast over partitions ----
        recip = sb.tile([1, HPC * Q], F32, tag="recip")
        nc.vector.reciprocal(recip[:], sums_f.rearrange("p h t -> p (h t)"))
        ps_bc = ps.tile([128, HPC * Q], F32, tag="S1", bufs=2)
        nc.tensor.matmul(ps_bc[:], ones_row, recip[0:1, :], start=True, stop=True)
        bc = sb.tile([128, HPC * Q], F32, tag="bcsb")
        nc.vector.tensor_copy(bc[:], ps_bc[:])
        attn = sb.tile([128, HPC * Q], F16, tag="attn")
        nc.vector.tensor_mul(attn[:], ps_at.rearrange("p h t -> p (h t)"), bc[:])

        # ---- o_proj partial: out[8, 5120], shipped out per piece on the
        # scalar ring so only the last piece's store is on the tail ----
        out_sb = sb.tile([Q, HIDDEN], F32, tag="outsb")
        H2 = WO_P // 2
        tasks = [(p * WO_P, WO_P) for p in range(NPIECE - 1)]
        tasks += [((NPIECE - 1) * WO_P, H2), ((NPIECE - 1) * WO_P + H2, H2)]
        done = 0
        for (n0, w) in tasks:
            ps_o = ps.tile([Q, WO_P], F32, tag="PO", bufs=2)
            for h in range(HPC):
                nc.tensor.matmul(
                    ps_o[0:Q, 0:w],
                    attn[:, h * Q:(h + 1) * Q],
                    wo_sb[:, n0 // WO_P, h, n0 % WO_P:n0 % WO_P + w],
                    start=(h == 0),
                    stop=(h == HPC - 1),
                )
            nc.vector.tensor_copy(out_sb[0:Q, n0:n0 + w], ps_o[0:Q, 0:w])
            # ship accumulated columns once >= 1024 are ready (or at the end)
            if (n0 + w) - done >= 2 * WO_P or (n0 + w) == HIDDEN:
                nc.scalar.dma_start(
                    out_d.ap()[:, done:n0 + w], out_sb[0:Q, done:n0 + w])
                done = n0 + w

    nc.compile()
    return nc


def _get_program(pos, winners):
    key = (tuple(pos), tuple(winners))
    if key not in _CACHE:
        _CACHE[key] = _build_program(pos, winners)
    return _CACHE[key]


def _prep_inputs(input_pos, hidden_states, attention_mask, W_pack, W_o,
                 k_cache, v_cache):
    """Host-side sharding: returns (in_maps, pos, winners)."""
    pos = [int(p) for p in np.asarray(input_pos).reshape(-1)]
    last = {}
    for t, p in enumerate(pos):
        last[p] = t
    winners = [last[p] == t for t, p in enumerate(pos)]

    hs = np.asarray(hidden_states, dtype=np.float32).reshape(Q, HIDDEN)
    # hsT[p, kc*8+t] = hs[t, kc*128+p]; final column = fp16 ones
    hsT = np.ones((128, KC * Q + 1), dtype=np.float16)
    hsT[:, 0:KC * Q] = hs.T.reshape(KC, 128, Q).transpose(1, 0, 2).reshape(128, KC * Q)

    Wp = np.asarray(W_pack, dtype=np.float32)
    Wo = np.asarray(W_o, dtype=np.float32)
    kc_all = np.asarray(k_cache, dtype=np.float32)[0]   # [40, 2048, 128]
    vc_all = np.asarray(v_cache, dtype=np.float32)[0]
    mask = np.asarray(attention_mask, dtype=np.float32)
    mrows = mask[:, pos, :]                              # [40, 8, 2048]

    scale = np.float32(1.0 / math.sqrt(HD))

    in_maps = []
    for c in range(NCORES):
        r0 = c * HPC * HD
        r1 = (c + 1) * HPC * HD
        wsh = np.concatenate(
            [Wp[r0:r1] * scale, Wp[HIDDEN + r0:HIDDEN + r1],
             Wp[2 * HIDDEN + r0:2 * HIDDEN + r1]], axis=0)   # [1920, 5120]
        # [128 p, 40 kc, 1920 m]
        wpT = np.ascontiguousarray(
            wsh.T.reshape(KC, 128, MQKV).transpose(1, 0, 2)).astype(np.float16)
        heads = slice(c * HPC, (c + 1) * HPC)
        # [128 d, 5 h, 2048 pos]  (old cache as-is: winner columns are
        # neutralized via the -1e30 mask, not by insertion)
        kcT = np.ascontiguousarray(
            kc_all[heads].transpose(2, 0, 1)).astype(np.float16)
        # [128 p, 5 h, 16 c, 128 d]
        vcc = np.ascontiguousarray(
            vc_all[heads].reshape(HPC, NPOS, 128, HD)
            .transpose(2, 0, 1, 3)).astype(np.float16)
        # constants blob + fp16 maskT
        cb = np.zeros((128, CB_N), dtype=np.float32)
        # maskT[p, h, cj, t] = mrows[own_h, t, cj*128+p]; winner columns
        # -> -60000 (their exp terms come from the corr side path instead)
        mT = mrows[heads].reshape(HPC, Q, NPOS, 128).transpose(3, 0, 2, 1).copy()
        for t in range(Q):
            if winners[t]:
                mT[pos[t] % 128, :, pos[t] // 128, :] = np.float32(MASK_NEG)
        mkT = mT.reshape(128, -1).astype(np.float16)
        cb[0:Q, CB_EYE8:CB_EYE8 + Q] = np.eye(Q, dtype=np.float32)
        # maskN[j, h, t] = mrows[own_h, t, pos_j]; -1e30 for duplicate losers
        mN = mrows[heads][:, :, pos].transpose(2, 0, 1).copy()
        for j in range(Q):
            if not winners[j]:
                mN[j] = np.float32(-1e30)
        cb[0:Q, CB_MASKN:CB_MASKN + HPC * Q] = mN.reshape(Q, -1)
        cb[0:1, CB_ONES:CB_ONES + 128] = 1.0
        cb[0:Q, CB_ONES8:CB_ONES8 + 1] = 1.0
        # [128 p, 10 piece, 5 h, 512 n]
        wo = np.ascontiguousarray(
            Wo[:, r0:r1].reshape(NPIECE, WO_P, HPC, 128)
            .transpose(3, 0, 2, 1)).astype(np.float16)
        in_maps.append({
            "hsT": hsT, "wpT": wpT, "kcT": kcT, "vc": vcc, "cb": cb,
            "mkT": mkT, "wo": wo,
        })
    return in_maps, pos, winners


def kernel(input_pos, hidden_states, attention_mask, W_pack, W_o,
           k_cache, v_cache, _profile=False):
    in_maps, pos, winners = _prep_inputs(
        input_pos, hidden_states, attention_mask, W_pack, W_o, k_cache, v_cache)
    nc = _get_program(pos, winners)
    res = run_bass_kernel_spmd(nc, in_maps, list(range(NCORES)), trace=_profile)
    out = np.zeros((Q, HIDDEN), dtype=np.float64)
    for r in res.results:
        out += r["out"].astype(np.float64)
    full = out.astype(np.float32).reshape(1, Q, HIDDEN)
    if _profile:
        return full, res
    return full
